# revision 3
# baseline (speedup 1.0000x reference)
"""Trainium2 Bass kernel for nn_Blur: depthwise 4x4 FIR conv, pad=2.

out[b,c,h',w'] = sum_{i,j} wf[i,j] * xpad[b,c,h'+i,w'+j],  wf = flip(kernel)
x: [8,256,256,256] f32, kernel: [4,4] f32 -> out: [8,256,257,257] f32

Strategy: pure data parallel over batch (8 cores, 1 batch elem each).
Per core, the full 2D conv is done on the TensorEngine as 4 banded-matrix
matmuls (one per kernel column j) accumulating in PSUM:
    psum[h', w'] += sum_h B_j[h,h'] * xpad_w[h, w'+j]
with B_j[h,h'] = wf[h-h'+2, j] built on the host from the runtime kernel.
float32r dtype gives full PE rate (1 cyc/row at N>=256, even N required);
rel err ~2e-4 which is far inside the 2e-2 gate.
"""

import numpy as np

_C, _H, _W = 256, 256, 256
_HO, _WO = 257, 257
_NCORES = 8
# (hp0, Mv, hlo, Kv): output rows [hp0, hp0+Mv), contraction rows [hlo, hlo+Kv)
_TILES = [(0, 125, 0, 126), (125, 125, 123, 128), (250, 7, 248, 8)]
_NW = 262  # padded width in SBUF: 2 zero | 256 data | 4 zero
_NMM = 258  # matmul free dim (257 outputs + 1 garbage col), must be even


def _build_bands(kern):
    wf = np.ascontiguousarray(np.asarray(kern, np.float32)[::-1, ::-1])
    bands = np.zeros((128, 3, 4, 125), np.float32)
    for v, (hp0, Mv, hlo, Kv) in enumerate(_TILES):
        for j in range(4):
            for hr in range(Kv):
                h = hlo + hr
                for mr in range(Mv):
                    i = h - (hp0 + mr) + 2
                    if 0 <= i < 4:
                        bands[hr, v, j, mr] = wf[i, j]
    return bands


_NC_CACHE = {}


def _build_nc():
    if "nc" in _NC_CACHE:
        return _NC_CACHE["nc"]
    import concourse.bacc as bacc
    import concourse.mybir as mybir
    import concourse.tile as tile

    nc = bacc.Bacc()
    x_d = nc.declare_dram_parameter("x", [_C, _H, _W], mybir.dt.float32r, isOutput=False)
    b_d = nc.declare_dram_parameter(
        "bands", [128, 3, 4, 125], mybir.dt.float32r, isOutput=False
    )
    o_d = nc.declare_dram_parameter("out", [_C, _HO, _WO], mybir.dt.float32, isOutput=True)
    z_d = nc.declare_dram_parameter("zpad", [128, 8], mybir.dt.float32r, isOutput=False)

    NBX = 12  # x-tile ring depth
    NBO = 12  # out-tile ring depth
    NBP = 8  # psum banks
    with tile.TileContext(nc) as tc:
        with (
            tc.tile_pool(name="sb", bufs=1) as pool,
            tc.tile_pool(name="ps", bufs=1, space="PSUM") as pp,
        ):
            band_sb = pool.tile([128, 3, 4, 125], mybir.dt.float32r, tag="bands")
            nc.sync.dma_start(out=band_sb[:], in_=b_d[:])

            xts = []
            for i in range(NBX):
                t = pool.tile([128, _NW], mybir.dt.float32r, tag=f"xt{i}", name=f"xt{i}")
                nc.sync.dma_start(out=t[:, 0:2], in_=z_d[:, 0:2])
                nc.sync.dma_start(out=t[:, 258:_NW], in_=z_d[:, 0:4])
                xts.append(t)
            oss = [
                pool.tile([128, _NMM], mybir.dt.float32, tag=f"os{i}", name=f"os{i}") for i in range(NBO)
            ]
            pss = [
                pp.tile([128, _NMM], mybir.dt.float32, tag=f"ps{i}", name=f"ps{i}") for i in range(NBP)
            ]

            it = 0
            for c in range(_C):
                for v, (hp0, Mv, hlo, Kv) in enumerate(_TILES):
                    xt = xts[it % NBX]
                    ps = pss[it % NBP]
                    osb = oss[it % NBO]
                    nc.sync.dma_start(out=xt[0:Kv, 2:258], in_=x_d[c, hlo : hlo + Kv, :])
                    for j in range(4):
                        nc.tensor.matmul(
                            ps[0:Mv, 0:_NMM],
                            band_sb[0:Kv, v, j, 0:Mv],
                            xt[0:Kv, j : j + _NMM],
                            start=(j == 0),
                            stop=(j == 3),
                        )
                    if it % 2 == 0:
                        nc.vector.tensor_copy(osb[0:Mv, 0:_WO], ps[0:Mv, 0:_WO])
                    else:
                        nc.scalar.copy(osb[0:Mv, 0:_WO], ps[0:Mv, 0:_WO])
                    nc.sync.dma_start(
                        out=o_d[c, hp0 : hp0 + Mv, :], in_=osb[0:Mv, 0:_WO]
                    )
                    it += 1
    nc.finalize()
    _NC_CACHE["nc"] = nc
    return nc


def _run(x, kern, trace=False):
    from concourse.bass_utils import run_bass_kernel_spmd

    x = np.asarray(x, dtype=np.float32)
    bands = _build_bands(kern)
    nc = _build_nc()
    zpad = np.zeros((128, 8), np.float32)
    in_maps = [
        {"x": np.ascontiguousarray(x[b]), "bands": bands, "zpad": zpad}
        for b in range(_NCORES)
    ]
    res = run_bass_kernel_spmd(nc, in_maps, list(range(_NCORES)), trace=trace)
    out = np.stack(
        [np.asarray(res.results[i]["out"]) for i in range(_NCORES)], axis=0
    ).astype(np.float32)
    return out, res


def kernel(x, kernel):
    out, _ = _run(x, kernel, trace=False)
    return out


# revision 4
# speedup vs baseline: 1.4793x; 1.4793x over previous
"""Trainium2 Bass kernel for nn_Blur: depthwise 4x4 FIR conv, pad=2.

out[b,c,h',w'] = sum_{i,j} wf[i,j] * xpad[b,c,h'+i,w'+j],  wf = flip(kernel)
x: [8,256,256,256] f32, kernel: [4,4] f32 -> out: [8,256,257,257] f32

Strategy: pure data parallel over batch (8 cores, 1 batch elem each).
Per core, the full 2D conv runs on the TensorEngine as 4 banded-matrix
matmuls (one per kernel column j) accumulating in PSUM:
    psum[h', w'] += sum_h B_j[h,h'] * xpad_w[h, w'+j]
with B_j[h,h'] = wf[h-h'+2, j] built on the host from the runtime kernel.
float32r dtype gives full PE rate (1 cyc/row at N>=256, even N required);
rel err ~2e-4, far inside the 2e-2 gate.

DMA layout: 8 channels batched per transfer (~1 MB per dma_start) to
amortize the ~0.8-2us per-DMA completion latency; input loads issue on
the SP HWDGE ring (nc.sync), output stores on the ACT ring (nc.scalar)
so the two FIFO rings overlap.
"""

import numpy as np

_C, _H, _W = 256, 256, 256
_HO, _WO = 257, 257
_NCORES = 8
# (hp0, Mv, hlo, Kv): output rows [hp0, hp0+Mv), contraction rows [hlo, hlo+Kv)
_TILES = [(0, 125, 0, 126), (125, 125, 123, 128), (250, 7, 248, 8)]
_NW = 262  # padded width in SBUF: 2 zero | 256 data | 4 zero
_NMM = 258  # matmul free dim (257 outputs + 1 garbage col), must be even
_CB = 8  # channels per DMA batch / psum rotation


def _build_bands(kern):
    wf = np.ascontiguousarray(np.asarray(kern, np.float32)[::-1, ::-1])
    bands = np.zeros((128, 3, 4, 125), np.float32)
    for v, (hp0, Mv, hlo, Kv) in enumerate(_TILES):
        for j in range(4):
            for hr in range(Kv):
                h = hlo + hr
                for mr in range(Mv):
                    i = h - (hp0 + mr) + 2
                    if 0 <= i < 4:
                        bands[hr, v, j, mr] = wf[i, j]
    return bands


_NC_CACHE = {}


def _build_nc():
    if "nc" in _NC_CACHE:
        return _NC_CACHE["nc"]
    import concourse.bacc as bacc
    import concourse.mybir as mybir
    import concourse.tile as tile

    nc = bacc.Bacc()
    x_d = nc.declare_dram_parameter("x", [_C, _H, _W], mybir.dt.float32r, isOutput=False)
    b_d = nc.declare_dram_parameter(
        "bands", [128, 3, 4, 125], mybir.dt.float32r, isOutput=False
    )
    o_d = nc.declare_dram_parameter("out", [_C, _HO, _WO], mybir.dt.float32, isOutput=True)
    z_d = nc.declare_dram_parameter("zpad", [128, _CB, 4], mybir.dt.float32r, isOutput=False)

    NBX = 4  # x-tile ring depth (each tile holds a whole channel-group's rows)
    NBO = 4  # out-tile ring depth
    NBP = 8  # psum banks: one per channel within a group
    with tile.TileContext(nc) as tc:
        with (
            tc.tile_pool(name="sb", bufs=1) as pool,
            tc.tile_pool(name="ps", bufs=1, space="PSUM") as pp,
        ):
            band_sb = pool.tile([128, 3, 4, 125], mybir.dt.float32r, tag="bands")
            nc.sync.dma_start(out=band_sb[:], in_=b_d[:])

            xts = []
            for i in range(NBX):
                t = pool.tile(
                    [128, _CB, _NW], mybir.dt.float32r, tag=f"xt{i}", name=f"xt{i}"
                )
                nc.sync.dma_start(out=t[:, :, 0:2], in_=z_d[:, :, 0:2])
                nc.sync.dma_start(out=t[:, :, 258:_NW], in_=z_d[:, :, 0:4])
                xts.append(t)
            oss = [
                pool.tile(
                    [128, _CB, _NMM], mybir.dt.float32, tag=f"os{i}", name=f"os{i}"
                )
                for i in range(NBO)
            ]
            pss = [
                pp.tile([128, _NMM], mybir.dt.float32, tag=f"ps{i}", name=f"ps{i}")
                for i in range(NBP)
            ]

            it = 0
            for c0 in range(0, _C, _CB):
                for v, (hp0, Mv, hlo, Kv) in enumerate(_TILES):
                    xt = xts[it % NBX]
                    osb = oss[it % NBO]
                    nc.sync.dma_start(
                        out=xt[0:Kv, :, 2:258],
                        in_=x_d[c0 : c0 + _CB, hlo : hlo + Kv, :].rearrange(
                            "c h w -> h c w"
                        ),
                    )
                    for cc in range(_CB):
                        ps = pss[cc]
                        for j in range(4):
                            nc.tensor.matmul(
                                ps[0:Mv, 0:_NMM],
                                band_sb[0:Kv, v, j, 0:Mv],
                                xt[0:Kv, cc, j : j + _NMM],
                                start=(j == 0),
                                stop=(j == 3),
                            )
                        if cc % 2 == 0:
                            nc.vector.tensor_copy(
                                osb[0:Mv, cc, 0:_WO], ps[0:Mv, 0:_WO]
                            )
                        else:
                            nc.scalar.copy(osb[0:Mv, cc, 0:_WO], ps[0:Mv, 0:_WO])
                    nc.scalar.dma_start(
                        out=o_d[c0 : c0 + _CB, hp0 : hp0 + Mv, :].rearrange(
                            "c h w -> h c w"
                        ),
                        in_=osb[0:Mv, :, 0:_WO],
                    )
                    it += 1
    nc.finalize()
    _NC_CACHE["nc"] = nc
    return nc


def _run(x, kern, trace=False):
    from concourse.bass_utils import run_bass_kernel_spmd

    x = np.asarray(x, dtype=np.float32)
    bands = _build_bands(kern)
    nc = _build_nc()
    zpad = np.zeros((128, _CB, 4), np.float32)
    in_maps = [
        {"x": np.ascontiguousarray(x[b]), "bands": bands, "zpad": zpad}
        for b in range(_NCORES)
    ]
    res = run_bass_kernel_spmd(nc, in_maps, list(range(_NCORES)), trace=trace)
    out = np.stack(
        [np.asarray(res.results[i]["out"]) for i in range(_NCORES)], axis=0
    ).astype(np.float32)
    return out, res


def kernel(x, kernel):
    out, _ = _run(x, kernel, trace=False)
    return out


# revision 5
# speedup vs baseline: 1.6925x; 1.1441x over previous
"""Trainium2 Bass kernel for nn_Blur: depthwise 4x4 FIR conv, pad=2.

out[b,c,h',w'] = sum_{i,j} wf[i,j] * xpad[b,c,h'+i,w'+j],  wf = flip(kernel)
x: [8,256,256,256] f32, kernel: [4,4] f32 -> out: [8,256,257,257] f32

Strategy: pure data parallel over batch (8 cores, 1 batch elem each).
Per core, the full 2D conv runs on the TensorEngine as 4 banded-matrix
matmuls (one per kernel column j) accumulating in PSUM:
    psum[h', w'] += sum_h B_j[h,h'] * xpad_w[h, w'+j]
with B_j[h,h'] = wf[h-h'+2, j] built on the host from the runtime kernel.
float32r dtype gives full PE rate (1 cyc/row at N>=256, even N required);
rel err ~2e-4, far inside the 2e-2 gate.

DMA layout: 8 channels batched per transfer (~1 MB per dma_start) to
amortize the ~0.8-2us per-DMA completion latency; input loads issue on
the SP HWDGE ring (nc.sync), output stores on the ACT ring (nc.scalar)
so the two FIFO rings overlap.
"""

import numpy as np

_C, _H, _W = 256, 256, 256
_HO, _WO = 257, 257
_NCORES = 8
# (hp0, Mv, hlo, Kv): output rows [hp0, hp0+Mv), contraction rows [hlo, hlo+Kv)
_TILES = [(0, 125, 0, 126), (125, 125, 123, 128), (250, 7, 248, 8)]
_NW = 262  # padded width in SBUF: 2 zero | 256 data | 4 zero
_NMM = 258  # matmul free dim (257 outputs + 1 garbage col), must be even
_CB = 8  # channels per DMA batch / psum rotation


def _build_bands(kern):
    wf = np.ascontiguousarray(np.asarray(kern, np.float32)[::-1, ::-1])
    bands = np.zeros((128, 3, 4, 125), np.float32)
    for v, (hp0, Mv, hlo, Kv) in enumerate(_TILES):
        for j in range(4):
            for hr in range(Kv):
                h = hlo + hr
                for mr in range(Mv):
                    i = h - (hp0 + mr) + 2
                    if 0 <= i < 4:
                        bands[hr, v, j, mr] = wf[i, j]
    return bands


_NC_CACHE = {}


def _build_nc():
    if "nc" in _NC_CACHE:
        return _NC_CACHE["nc"]
    import concourse.bacc as bacc
    import concourse.mybir as mybir
    import concourse.tile as tile

    nc = bacc.Bacc()
    x_d = nc.declare_dram_parameter("x", [_C, _H, _W], mybir.dt.float32r, isOutput=False)
    b_d = nc.declare_dram_parameter(
        "bands", [128, 3, 4, 125], mybir.dt.float32r, isOutput=False
    )
    o_d = nc.declare_dram_parameter("out", [_C, _HO, _WO], mybir.dt.float32, isOutput=True)
    z_d = nc.declare_dram_parameter("zpad", [128, _CB, 4], mybir.dt.float32r, isOutput=False)

    NBX = 4  # x-tile ring depth (each tile holds a whole channel-group's rows)
    NBO = 4  # out-tile ring depth
    NBP = 8  # psum banks: one per channel within a group
    with tile.TileContext(nc) as tc:
        with (
            tc.tile_pool(name="sb", bufs=1) as pool,
            tc.tile_pool(name="ps", bufs=1, space="PSUM") as pp,
        ):
            band_sb = pool.tile([128, 3, 4, 125], mybir.dt.float32r, tag="bands")
            nc.sync.dma_start(out=band_sb[:], in_=b_d[:])

            xts = []
            for i in range(NBX):
                t = pool.tile(
                    [128, _CB, _NW], mybir.dt.float32r, tag=f"xt{i}", name=f"xt{i}"
                )
                nc.sync.dma_start(out=t[:, :, 0:2], in_=z_d[:, :, 0:2])
                nc.sync.dma_start(out=t[:, :, 258:_NW], in_=z_d[:, :, 0:4])
                xts.append(t)
            oss = [
                pool.tile(
                    [128, _CB, _NMM], mybir.dt.float32, tag=f"os{i}", name=f"os{i}"
                )
                for i in range(NBO)
            ]
            pss = [
                pp.tile([128, _NMM], mybir.dt.float32, tag=f"ps{i}", name=f"ps{i}")
                for i in range(NBP)
            ]

            it = 0
            for c0 in range(0, _C, _CB):
                for v, (hp0, Mv, hlo, Kv) in enumerate(_TILES):
                    xt = xts[it % NBX]
                    osb = oss[it % NBO]
                    nc.sync.dma_start(
                        out=xt[0:Kv, :, 2:258],
                        in_=x_d[c0 : c0 + _CB, hlo : hlo + Kv, :].rearrange(
                            "c h w -> h c w"
                        ),
                    )
                    for cc in range(_CB):
                        ps = pss[cc]
                        for j in range(4):
                            nc.tensor.matmul(
                                ps[0:Mv, 0:_NMM],
                                band_sb[0:Kv, v, j, 0:Mv],
                                xt[0:Kv, cc, j : j + _NMM],
                                start=(j == 0),
                                stop=(j == 3),
                            )
                        if cc % 2 == 0:
                            nc.vector.tensor_copy(
                                osb[0:Mv, cc, 0:_WO], ps[0:Mv, 0:_WO]
                            )
                        else:
                            nc.scalar.copy(osb[0:Mv, cc, 0:_WO], ps[0:Mv, 0:_WO])
                    nc.gpsimd.dma_start(
                        out=o_d[c0 : c0 + _CB, hp0 : hp0 + Mv, :].rearrange(
                            "c h w -> h c w"
                        ),
                        in_=osb[0:Mv, :, 0:_WO],
                    )
                    it += 1
    nc.finalize()
    _NC_CACHE["nc"] = nc
    return nc


def _run(x, kern, trace=False):
    from concourse.bass_utils import run_bass_kernel_spmd

    x = np.asarray(x, dtype=np.float32)
    bands = _build_bands(kern)
    nc = _build_nc()
    zpad = np.zeros((128, _CB, 4), np.float32)
    in_maps = [
        {"x": np.ascontiguousarray(x[b]), "bands": bands, "zpad": zpad}
        for b in range(_NCORES)
    ]
    res = run_bass_kernel_spmd(nc, in_maps, list(range(_NCORES)), trace=trace)
    out = np.stack(
        [np.asarray(res.results[i]["out"]) for i in range(_NCORES)], axis=0
    ).astype(np.float32)
    return out, res


def kernel(x, kernel):
    out, _ = _run(x, kernel, trace=False)
    return out


# revision 8
# speedup vs baseline: 2.0170x; 1.1917x over previous
"""Trainium2 Bass kernel for nn_Blur: depthwise 4x4 FIR conv, pad=2.

out[b,c,h',w'] = sum_{i,j} wf[i,j] * xpad[b,c,h'+i,w'+j],  wf = flip(kernel)
x: [8,256,256,256] f32, kernel: [4,4] f32 -> out: [8,256,257,257] f32

Strategy: pure data parallel over batch (8 cores, 1 batch elem each).
Per core, the full 2D conv runs on the TensorEngine as 4 banded-matrix
matmuls (one per kernel column j) accumulating in PSUM:
    psum[h', w'] += sum_h B_j[h,h'] * xpad_w[h, w'+j]
with B_j[h,h'] = wf[h-h'+2, j] built on the host from the runtime kernel.
float32r dtype gives full PE rate (1 cyc/row at N>=256, even N required);
rel err ~2e-4, far inside the 2e-2 gate.

DMA layout: 8 channels batched per transfer (~1 MB per dma_start) to
amortize the ~0.8-2us per-DMA completion latency; input loads issue on
the SP HWDGE ring (nc.sync), output stores on the ACT ring (nc.scalar)
so the two FIFO rings overlap.
"""

import numpy as np

_C, _H, _W = 256, 256, 256
_HO, _WO = 257, 257
_NCORES = 8
# (hp0, Mv, hlo, Kv): output rows [hp0, hp0+Mv), contraction rows [hlo, hlo+Kv)
_TILES = [(0, 125, 0, 126), (125, 125, 123, 128), (250, 7, 248, 8)]
_NW = 262  # padded width in SBUF: 2 zero | 256 data | 4 zero
_NMM = 258  # matmul free dim (257 outputs + 1 garbage col), must be even
_OPAD = 264  # padded output row pitch in DRAM (1056B, 32B-aligned)
_CB = 8  # channels per DMA batch / psum rotation


def _build_bands(kern):
    wf = np.ascontiguousarray(np.asarray(kern, np.float32)[::-1, ::-1])
    bands = np.zeros((128, 3, 4, 125), np.float32)
    for v, (hp0, Mv, hlo, Kv) in enumerate(_TILES):
        for j in range(4):
            for hr in range(Kv):
                h = hlo + hr
                for mr in range(Mv):
                    i = h - (hp0 + mr) + 2
                    if 0 <= i < 4:
                        bands[hr, v, j, mr] = wf[i, j]
    return bands


_NC_CACHE = {}


def _build_nc():
    if "nc" in _NC_CACHE:
        return _NC_CACHE["nc"]
    import concourse.bacc as bacc
    import concourse.mybir as mybir
    import concourse.tile as tile

    nc = bacc.Bacc()
    x_d = nc.declare_dram_parameter("x", [_C, _H, _W], mybir.dt.float32r, isOutput=False)
    b_d = nc.declare_dram_parameter(
        "bands", [128, 3, 4, 125], mybir.dt.float32r, isOutput=False
    )
    o_d = nc.declare_dram_parameter("out", [_C, _HO, _OPAD], mybir.dt.float32, isOutput=True)
    z_d = nc.declare_dram_parameter("zpad", [128, _CB, 4], mybir.dt.float32r, isOutput=False)

    NBX = 4  # x-tile ring depth (each tile holds a whole channel-group's rows)
    NBO = 4  # out-tile ring depth
    NBP = 8  # psum banks: one per channel within a group
    with tile.TileContext(nc) as tc:
        with (
            tc.tile_pool(name="sb", bufs=1) as pool,
            tc.tile_pool(name="ps", bufs=1, space="PSUM") as pp,
        ):
            band_sb = pool.tile([128, 3, 4, 125], mybir.dt.float32r, tag="bands")
            nc.sync.dma_start(out=band_sb[:], in_=b_d[:])

            xts = []
            for i in range(NBX):
                t = pool.tile(
                    [128, _CB, _NW], mybir.dt.float32r, tag=f"xt{i}", name=f"xt{i}"
                )
                nc.sync.dma_start(out=t[:, :, 0:2], in_=z_d[:, :, 0:2])
                nc.sync.dma_start(out=t[:, :, 258:_NW], in_=z_d[:, :, 0:4])
                xts.append(t)
            oss = [
                pool.tile(
                    [128, _CB, _OPAD], mybir.dt.float32, tag=f"os{i}", name=f"os{i}"
                )
                for i in range(NBO)
            ]
            pss = [
                pp.tile([128, _NMM], mybir.dt.float32, tag=f"ps{i}", name=f"ps{i}")
                for i in range(NBP)
            ]

            it = 0
            for c0 in range(0, _C, _CB):
                for v, (hp0, Mv, hlo, Kv) in enumerate(_TILES):
                    xt = xts[it % NBX]
                    osb = oss[it % NBO]
                    nc.sync.dma_start(
                        out=xt[0:Kv, :, 2:258],
                        in_=x_d[c0 : c0 + _CB, hlo : hlo + Kv, :].rearrange(
                            "c h w -> h c w"
                        ),
                    )
                    for cc in range(_CB):
                        ps = pss[cc]
                        for j in range(4):
                            nc.tensor.matmul(
                                ps[0:Mv, 0:_NMM],
                                band_sb[0:Kv, v, j, 0:Mv],
                                xt[0:Kv, cc, j : j + _NMM],
                                start=(j == 0),
                                stop=(j == 3),
                            )
                        if cc % 2 == 0:
                            nc.vector.tensor_copy(
                                osb[0:Mv, cc, 0:_WO], ps[0:Mv, 0:_WO]
                            )
                        else:
                            nc.scalar.copy(osb[0:Mv, cc, 0:_WO], ps[0:Mv, 0:_WO])
                    nc.gpsimd.dma_start(
                        out=o_d[c0 : c0 + _CB, hp0 : hp0 + Mv, :].rearrange(
                            "c h w -> h c w"
                        ),
                        in_=osb[0:Mv, :, 0:_OPAD],
                    )
                    it += 1
    nc.finalize()
    _NC_CACHE["nc"] = nc
    return nc


def _run(x, kern, trace=False):
    from concourse.bass_utils import run_bass_kernel_spmd

    x = np.asarray(x, dtype=np.float32)
    bands = _build_bands(kern)
    nc = _build_nc()
    zpad = np.zeros((128, _CB, 4), np.float32)
    in_maps = [
        {"x": np.ascontiguousarray(x[b]), "bands": bands, "zpad": zpad}
        for b in range(_NCORES)
    ]
    res = run_bass_kernel_spmd(nc, in_maps, list(range(_NCORES)), trace=trace)
    out = np.stack(
        [np.asarray(res.results[i]["out"])[:, :, : _WO] for i in range(_NCORES)],
        axis=0,
    ).astype(np.float32)
    return out, res


def kernel(x, kernel):
    out, _ = _run(x, kernel, trace=False)
    return out
